# revision 6
# baseline (speedup 1.0000x reference)
"""GQA attention (S=2048, D=4096, 32 Q heads / 8 KV heads, RoPE, full attn)
distributed over 8 Trainium2 NeuronCores.

Strategy (tensor-parallel by heads, Megatron-style with an AllGather before
the output projection instead of an AllReduce after it):
  - core c owns Q heads 4c..4c+3 and KV head c (GQA groups align with cores).
  - projections computed as transposed GEMMs: QT/KT [chan, tok] directly
    usable by the scores matmul; V via VT + PE transposes.
  - RoPE folded into two PE "mix" matmuls over host-deinterleaved channels
    (evens then odds per head), scale folded into wq on the host.
  - scores computed transposed, ST = KT.T @ QT -> [k, q]: softmax normalizer
    via DVE partial sums + one ones-matmul; exp on ScalarE (f32, near-exact);
    PV matmul takes V as stationary operand and E as moving operand.
  - attnT [512, 2048] normalized, AllGathered per 512-token q-chunk (4
    pipelined AllGathers), then each core computes a 512-row slice of the
    transposed output projection finalT = woT.T @ attnT_full.
  - all matmuls run in float32r (bf16-speed, ~1.5e-4 matmul error).

Host side only reshapes/transposes/pads and concatenates outputs.
"""
import sys

import numpy as np

for _p in ("/root/.axon_site/_ro/trn_rl_repo", "/opt/trn_rl_repo"):
    if _p not in sys.path:
        sys.path.append(_p)

import concourse.bass as bass
import concourse.tile as tile
from concourse import mybir
from concourse.bass_utils import run_bass_kernel_spmd

N_CORES = 8
S = 2048
D = 4096
HD = 128
N_QH = 4          # Q heads per core
N_KT = S // 128   # 16 k-tiles
N_TC = S // 512   # 4 token chunks
N_KC = D // 128   # 32 contraction tiles
F32 = mybir.dt.float32
F32R = mybir.dt.float32r

_NC_CACHE = {}


def _bc(ap):
    return ap.bitcast(F32R)


def _split_multi_waits(nc):
    """This container's walrus accepts only ONE sync-wait per instruction
    encoding; hoist extra waits onto fresh single-wait NoOps placed before
    the instruction on the same engine."""
    n = 0
    for fn in nc.m.functions:
        for bb in fn.blocks:
            new_insts = []
            changed = False
            for ins in bb.instructions:
                si = ins.sync_info
                waits = list(si.on_wait) if si is not None else []
                if len(waits) > 1:
                    for w in waits[:-1]:
                        n += 1
                        nop = mybir.InstNoOp(name=f"WSPL-{n}", ins=[], outs=[])
                        nop.engine = ins.engine
                        nop.sync_info = mybir.SyncInfo(on_wait=[w], on_update=[])
                        new_insts.append(nop)
                    si.on_wait = waits[-1:]
                    changed = True
                new_insts.append(ins)
            if changed:
                bb.instructions = new_insts
    return n


def _build():
    nc = bass.Bass()

    xt = nc.dram_tensor("xt", [D, S], F32R, kind="ExternalInput")
    wqt = nc.dram_tensor("wqt", [D, 512], F32R, kind="ExternalInput")
    wkt = nc.dram_tensor("wkt", [D, HD], F32R, kind="ExternalInput")
    wvt = nc.dram_tensor("wvt", [D, HD], F32R, kind="ExternalInput")
    wot = nc.dram_tensor("wot", [D, 512], F32R, kind="ExternalInput")
    cs1 = nc.dram_tensor("cs1", [HD, S], F32, kind="ExternalInput")
    cs2 = nc.dram_tensor("cs2", [HD, S], F32, kind="ExternalInput")
    mix1 = nc.dram_tensor("mix1", [HD, HD], F32R, kind="ExternalInput")
    mix2 = nc.dram_tensor("mix2", [HD, HD], F32R, kind="ExternalInput")
    onesc = nc.dram_tensor("onesc", [HD, 1], F32R, kind="ExternalInput")
    onesr = nc.dram_tensor("onesr", [1, HD], F32R, kind="ExternalInput")
    ident = nc.dram_tensor("ident", [HD, HD], F32R, kind="ExternalInput")
    out_ext = nc.dram_tensor("out", [512, S], F32, kind="ExternalOutput")

    ag_in = [
        nc.dram_tensor(f"agi{qc}", [512, 512], F32R) for qc in range(N_TC)
    ]
    ag_out = [
        nc.dram_tensor(f"ago{qc}", [D, 512], F32R, addr_space="Shared")
        for qc in range(N_TC)
    ]

    xt_r = xt.rearrange("(kc p) s -> kc p s", p=128)
    wqt_r = wqt.rearrange("(kc p) n -> kc p n", p=128)
    wkt_r = wkt.rearrange("(kc p) n -> kc p n", p=128)
    wvt_r = wvt.rearrange("(kc p) n -> kc p n", p=128)
    wot_r = wot.rearrange("(hk p) n -> hk p n", p=128)

    with tile.TileContext(nc) as tc:
        with (
            tc.tile_pool(name="const", bufs=1) as constp,
            tc.tile_pool(name="persist", bufs=1) as persist,
        ):
            # constants
            cs1_sb = constp.tile([HD, S], F32)
            cs2_sb = constp.tile([HD, S], F32)
            mix1_sb = constp.tile([HD, HD], F32R)
            mix2_sb = constp.tile([HD, HD], F32R)
            onesc_sb = constp.tile([HD, 1], F32R)
            onesr_sb = constp.tile([1, HD], F32R)
            ident_sb = constp.tile([HD, HD], F32R)
            nc.sync.dma_start(out=cs1_sb[:], in_=cs1[:])
            nc.sync.dma_start(out=cs2_sb[:], in_=cs2[:])
            nc.sync.dma_start(out=mix1_sb[:], in_=mix1[:])
            nc.sync.dma_start(out=mix2_sb[:], in_=mix2[:])
            nc.sync.dma_start(out=onesc_sb[:], in_=onesc[:])
            nc.sync.dma_start(out=onesr_sb[:], in_=onesr[:])
            nc.sync.dma_start(out=ident_sb[:], in_=ident[:])

            # persistent activations
            qt_sb = persist.tile([128, N_QH, S], F32R)   # QT_rope
            kt_sb = persist.tile([128, S], F32R)         # KT_rope
            v_sb = persist.tile([128, N_KT, HD], F32R)   # V [tok-in-tile, kt, chan]

            # ---------------- phase 1: projections + rope ----------------
            with (
                tc.tile_pool(name="wq", bufs=1) as wqp,
                tc.tile_pool(name="wkv", bufs=4) as wkvp,
                tc.tile_pool(name="xtp", bufs=6) as xtp,
                tc.tile_pool(name="uv", bufs=4) as uvp,
                tc.tile_pool(name="vt", bufs=2) as vtp,
                tc.tile_pool(name="p1q", bufs=1, space="PSUM") as p1q,
                tc.tile_pool(name="p1k", bufs=1, space="PSUM") as p1k,
                tc.tile_pool(name="p1r", bufs=1, space="PSUM") as p1r,
            ):
                wq_sb = wqp.tile([128, N_KC, 512], F32R)
                nc.sync.dma_start(out=wq_sb[:], in_=wqt_r[:].rearrange("kc p n -> p kc n"))

                for tcb in range(N_TC):
                    t0 = tcb * 512
                    qps = [
                        p1q.tile([128, 512], F32, name=f"qps{tcb}_{h}", tag=f"qps{h}")
                        for h in range(N_QH)
                    ]
                    kps = p1k.tile([128, 512], F32, name=f"kps{tcb}", tag="kps")
                    vtps = p1k.tile([128, 512], F32, name=f"vtps{tcb}", tag="vtps")
                    for kc in range(N_KC):
                        xt_t = xtp.tile([128, 512], F32R, name=f"xt{tcb}_{kc}", tag="xt")
                        nc.sync.dma_start(out=xt_t[:], in_=xt_r[kc, :, t0:t0 + 512])
                        wk_t = wkvp.tile([128, HD], F32R, name=f"wk{tcb}_{kc}", tag="wk")
                        wv_t = wkvp.tile([128, HD], F32R, name=f"wv{tcb}_{kc}", tag="wv")
                        nc.sync.dma_start(out=wk_t[:], in_=wkt_r[kc])
                        nc.sync.dma_start(out=wv_t[:], in_=wvt_r[kc])
                        st, sp = kc == 0, kc == N_KC - 1
                        for h in range(N_QH):
                            nc.tensor.matmul(
                                qps[h][:], wq_sb[:, kc, h * 128:(h + 1) * 128],
                                xt_t[:], start=st, stop=sp,
                            )
                        nc.tensor.matmul(kps[:], wk_t[:], xt_t[:], start=st, stop=sp)
                        nc.tensor.matmul(vtps[:], wv_t[:], xt_t[:], start=st, stop=sp)

                    # rope for Q heads and K of this token chunk
                    for h in range(N_QH + 1):
                        src = kps if h == N_QH else qps[h]
                        u_t = uvp.tile([128, 512], F32R, name=f"u{tcb}_{h}", tag="u")
                        v_t = uvp.tile([128, 512], F32R, name=f"v{tcb}_{h}", tag="v")
                        nc.vector.tensor_mul(u_t[:], src[:], cs1_sb[:, t0:t0 + 512])
                        nc.vector.tensor_mul(v_t[:], src[:], cs2_sb[:, t0:t0 + 512])
                        rps = p1r.tile([128, 512], F32, name=f"rps{tcb}_{h}", tag="rps")
                        nc.tensor.matmul(rps[:], mix1_sb[:], u_t[:], start=True, stop=False)
                        nc.tensor.matmul(rps[:], mix2_sb[:], v_t[:], start=False, stop=True)
                        if h == N_QH:
                            nc.vector.tensor_copy(kt_sb[:, t0:t0 + 512], rps[:])
                        else:
                            nc.vector.tensor_copy(qt_sb[:, h, t0:t0 + 512], rps[:])

                    # V for this token chunk: VT -> PE transpose -> V
                    vt_sb = vtp.tile([128, 512], F32R, name=f"vts{tcb}", tag="vts")
                    nc.vector.tensor_copy(vt_sb[:], vtps[:])
                    vtr = p1r.tile([128, 4, 128], F32R, name=f"vtr{tcb}", tag="vtr")
                    for j in range(4):
                        nc.tensor.transpose(
                            vtr[:, j, :], vt_sb[:, j * 128:(j + 1) * 128],
                            ident_sb[:],
                        )
                    nc.vector.tensor_copy(v_sb[:, tcb * 4:(tcb + 1) * 4, :], vtr[:])

            # ------------- phase 2+3: attention + AllGather per q-chunk -----
            # ------------- phase 4: wo GEMM per q-chunk ---------------------
            with (
                tc.tile_pool(name="wo", bufs=1) as wop,
                tc.tile_pool(name="ep", bufs=1) as ep,
                tc.tile_pool(name="zp", bufs=2) as zp,
                tc.tile_pool(name="np_", bufs=2) as np_,
                tc.tile_pool(name="agp", bufs=6) as agp,
                tc.tile_pool(name="fout", bufs=2) as foutp,
                tc.tile_pool(name="p2s", bufs=2, space="PSUM") as p2s,
                tc.tile_pool(name="p2pv", bufs=1, space="PSUM") as p2pv,
                tc.tile_pool(name="p2z", bufs=1, space="PSUM") as p2z,
                tc.tile_pool(name="p4f", bufs=1, space="PSUM") as p4f,
            ):
                wo_sb = wop.tile([128, N_KC, 512], F32R)
                nc.sync.dma_start(out=wo_sb[:], in_=wot_r[:].rearrange("hk p n -> p hk n"))

                for qc in range(N_TC):
                    q0 = qc * 512
                    for h in range(N_QH):
                        e_t = ep.tile([128, N_KT, 512], F32R, name=f"e{qc}_{h}", tag="e")
                        zpart = zp.tile([128, 512], F32, name=f"zp{qc}_{h}", tag="zpart")
                        pvps = p2pv.tile([128, 512], F32, name=f"pv{qc}_{h}", tag="pv")
                        for kt in range(N_KT):
                            k0 = kt * 128
                            stps = p2s.tile([128, 512], F32, name=f"st{qc}_{h}_{kt}", tag="st")
                            nc.tensor.matmul(
                                stps[:], kt_sb[:, k0:k0 + 128],
                                qt_sb[:, h, q0:q0 + 512], start=True, stop=True,
                            )
                            nc.scalar.activation(
                                out=e_t[:, kt, :], in_=stps[:],
                                func=mybir.ActivationFunctionType.Exp,
                            )
                            if kt == 0:
                                nc.vector.tensor_copy(zpart[:], e_t[:, 0, :])
                            else:
                                nc.vector.tensor_add(zpart[:], zpart[:], e_t[:, kt, :])
                            nc.tensor.matmul(
                                pvps[:], v_sb[:, kt, :], e_t[:, kt, :],
                                start=(kt == 0), stop=(kt == N_KT - 1),
                            )
                        zpr = zp.tile([128, 512], F32R, name=f"zpr{qc}_{h}", tag="zpr")
                        nc.vector.tensor_copy(zpr[:], zpart[:])
                        zps = p2z.tile([1, 512], F32, name=f"z{qc}_{h}", tag="zb")
                        nc.tensor.matmul(zps[:], onesc_sb[:], zpr[:], start=True, stop=True)
                        invz = zp.tile([1, 512], F32, name=f"iz{qc}_{h}", tag="invz")
                        nc.vector.reciprocal(invz[:], zps[:])
                        invzr = zp.tile([1, 512], F32R, name=f"izr{qc}_{h}", tag="invzr")
                        nc.vector.tensor_copy(invzr[:], invz[:])
                        bcps = p2z.tile([128, 512], F32, name=f"bc{qc}_{h}", tag="zb")
                        nc.tensor.matmul(bcps[:], onesr_sb[:], invzr[:], start=True, stop=True)
                        bc_sb = np_.tile([128, 512], F32, name=f"bcs{qc}_{h}", tag="bcs")
                        nc.vector.tensor_copy(bc_sb[:], bcps[:])
                        at_sb = np_.tile([128, 512], F32R, name=f"at{qc}_{h}", tag="at")
                        nc.vector.tensor_mul(at_sb[:], pvps[:], bc_sb[:])
                        nc.sync.dma_start(
                            out=ag_in[qc][h * 128:(h + 1) * 128, :], in_=at_sb[:]
                        )

                    nc.gpsimd.collective_compute(
                        "AllGather",
                        mybir.AluOpType.bypass,
                        replica_groups=[list(range(N_CORES))],
                        ins=[ag_in[qc][:].opt()],
                        outs=[ag_out[qc][:].opt()],
                    )

                    # wo GEMM for this q-chunk
                    fps = [
                        p4f.tile([128, 512], F32, name=f"f{qc}_{dt}", tag=f"f{dt}")
                        for dt in range(4)
                    ]
                    ago_r = ag_out[qc].rearrange("(hk p) n -> hk p n", p=128)
                    for hk in range(N_KC):
                        rhs_t = agp.tile([128, 512], F32R, name=f"ag{qc}_{hk}", tag="ag")
                        nc.sync.dma_start(out=rhs_t[:], in_=ago_r[hk])
                        for dt in range(4):
                            nc.tensor.matmul(
                                fps[dt][:], wo_sb[:, hk, dt * 128:(dt + 1) * 128],
                                rhs_t[:], start=(hk == 0), stop=(hk == N_KC - 1),
                            )
                    for dt in range(4):
                        f_sb = foutp.tile([128, 512], F32, name=f"fs{qc}_{dt}", tag="fs")
                        nc.vector.tensor_copy(f_sb[:], fps[dt][:])
                        nc.sync.dma_start(
                            out=out_ext[dt * 128:(dt + 1) * 128, q0:q0 + 512],
                            in_=f_sb[:],
                        )

    _split_multi_waits(nc)
    return nc


def _host_prep(x, cos, sin, wq, wk, wv, wo):
    scale = np.float32(HD ** -0.5)
    perm = np.concatenate([np.arange(0, HD, 2), np.arange(1, HD, 2)])

    xT = np.ascontiguousarray(x.T)
    cosT = np.ascontiguousarray(cos.T)
    sinT = np.ascontiguousarray(sin.T)
    cs1 = np.concatenate([cosT, sinT], axis=0)
    cs2 = np.concatenate([sinT, cosT], axis=0)

    m1 = np.zeros((HD, HD), np.float32)
    m1[np.arange(64), np.arange(64)] = 1.0
    m1[np.arange(64) + 64, np.arange(64)] = -1.0
    m2 = np.zeros((HD, HD), np.float32)
    m2[np.arange(64), np.arange(64) + 64] = 1.0
    m2[np.arange(64) + 64, np.arange(64) + 64] = 1.0

    shared = {
        "xt": xT,
        "cs1": cs1,
        "cs2": cs2,
        "mix1": m1,
        "mix2": m2,
        "onesc": np.ones((HD, 1), np.float32),
        "onesr": np.ones((1, HD), np.float32),
        "ident": np.eye(HD, dtype=np.float32),
    }
    in_maps = []
    for c in range(N_CORES):
        wq_c = wq[c * 512:(c + 1) * 512].reshape(N_QH, HD, D)[:, perm, :]
        wq_c = (wq_c.reshape(512, D) * scale)
        wk_c = wk[c * HD:(c + 1) * HD][perm, :]
        wv_c = wv[c * HD:(c + 1) * HD]
        wo_c = wo[c * 512:(c + 1) * 512]
        m = dict(shared)
        m["wqt"] = np.ascontiguousarray(wq_c.T)
        m["wkt"] = np.ascontiguousarray(wk_c.T)
        m["wvt"] = np.ascontiguousarray(wv_c.T)
        m["wot"] = np.ascontiguousarray(wo_c.T)
        in_maps.append(m)
    return in_maps


def kernel(x, cos, sin, wq, wk, wv, wo, _trace=False):
    x = np.asarray(x, np.float32)
    cos = np.asarray(cos, np.float32)
    sin = np.asarray(sin, np.float32)
    wq = np.asarray(wq, np.float32)
    wk = np.asarray(wk, np.float32)
    wv = np.asarray(wv, np.float32)
    wo = np.asarray(wo, np.float32)

    in_maps = _host_prep(x, cos, sin, wq, wk, wv, wo)
    if "nc" not in _NC_CACHE:
        _NC_CACHE["nc"] = _build()
    nc = _NC_CACHE["nc"]
    res = run_bass_kernel_spmd(
        nc, in_maps, core_ids=list(range(N_CORES)), trace=_trace
    )
    finalT = np.concatenate([res.results[c]["out"] for c in range(N_CORES)], axis=0)
    out = np.ascontiguousarray(finalT.T, dtype=np.float32)
    if _trace:
        kernel._last_exec_time_ns = res.exec_time_ns
        kernel._last_result = res
    return out


# revision 7
# speedup vs baseline: 1.0475x; 1.0475x over previous
"""GQA attention (S=2048, D=4096, 32 Q heads / 8 KV heads, RoPE, full attn)
distributed over 8 Trainium2 NeuronCores.

Strategy (tensor-parallel by heads, Megatron-style with an AllGather before
the output projection instead of an AllReduce after it):
  - core c owns Q heads 4c..4c+3 and KV head c (GQA groups align with cores).
  - projections computed as transposed GEMMs: QT/KT [chan, tok] directly
    usable by the scores matmul; V via VT + PE transposes.
  - RoPE folded into two PE "mix" matmuls over host-deinterleaved channels
    (evens then odds per head), scale folded into wq on the host.
  - scores computed transposed, ST = KT.T @ QT -> [k, q]: softmax normalizer
    via DVE partial sums + one ones-matmul; exp on ScalarE (f32, near-exact);
    PV matmul takes V as stationary operand and E as moving operand.
  - attnT [512, 2048] normalized, AllGathered per 512-token q-chunk (4
    pipelined AllGathers), then each core computes a 512-row slice of the
    transposed output projection finalT = woT.T @ attnT_full.
  - all matmuls run in float32r (bf16-speed, ~1.5e-4 matmul error).

Host side only reshapes/transposes/pads and concatenates outputs.
"""
import sys

import numpy as np

for _p in ("/root/.axon_site/_ro/trn_rl_repo", "/opt/trn_rl_repo"):
    if _p not in sys.path:
        sys.path.append(_p)

import concourse.bass as bass
import concourse.tile as tile
from concourse import mybir
from concourse.bass_utils import run_bass_kernel_spmd

N_CORES = 8
S = 2048
D = 4096
HD = 128
N_QH = 4          # Q heads per core
N_KT = S // 128   # 16 k-tiles
N_TC = S // 512   # 4 token chunks
N_KC = D // 128   # 32 contraction tiles
F32 = mybir.dt.float32
F32R = mybir.dt.float32r

_NC_CACHE = {}


def _bc(ap):
    return ap.bitcast(F32R)


def _split_multi_waits(nc):
    """This container's walrus accepts only ONE sync-wait per instruction
    encoding; hoist extra waits onto fresh single-wait NoOps placed before
    the instruction on the same engine."""
    n = 0
    for fn in nc.m.functions:
        for bb in fn.blocks:
            new_insts = []
            changed = False
            for ins in bb.instructions:
                si = ins.sync_info
                waits = list(si.on_wait) if si is not None else []
                if len(waits) > 1:
                    for w in waits[:-1]:
                        n += 1
                        nop = mybir.InstNoOp(name=f"WSPL-{n}", ins=[], outs=[])
                        nop.engine = ins.engine
                        nop.sync_info = mybir.SyncInfo(on_wait=[w], on_update=[])
                        new_insts.append(nop)
                    si.on_wait = waits[-1:]
                    changed = True
                new_insts.append(ins)
            if changed:
                bb.instructions = new_insts
    return n


def _build():
    nc = bass.Bass()

    xt = nc.dram_tensor("xt", [D, S], F32R, kind="ExternalInput")
    wqt = nc.dram_tensor("wqt", [D, 512], F32R, kind="ExternalInput")
    wkt = nc.dram_tensor("wkt", [D, HD], F32R, kind="ExternalInput")
    wvt = nc.dram_tensor("wvt", [D, HD], F32R, kind="ExternalInput")
    wot = nc.dram_tensor("wot", [D, 512], F32R, kind="ExternalInput")
    cs1 = nc.dram_tensor("cs1", [HD, S], F32, kind="ExternalInput")
    cs2 = nc.dram_tensor("cs2", [HD, S], F32, kind="ExternalInput")
    mix1 = nc.dram_tensor("mix1", [HD, HD], F32R, kind="ExternalInput")
    mix2 = nc.dram_tensor("mix2", [HD, HD], F32R, kind="ExternalInput")
    onesc = nc.dram_tensor("onesc", [HD, 1], F32R, kind="ExternalInput")
    onesr = nc.dram_tensor("onesr", [1, HD], F32R, kind="ExternalInput")
    ident = nc.dram_tensor("ident", [HD, HD], F32R, kind="ExternalInput")
    out_ext = nc.dram_tensor("out", [512, S], F32, kind="ExternalOutput")

    ag_in = [
        nc.dram_tensor(f"agi{qc}", [512, 512], F32R) for qc in range(N_TC)
    ]
    ag_out = [
        nc.dram_tensor(f"ago{qc}", [D, 512], F32R, addr_space="Shared")
        for qc in range(N_TC)
    ]

    xt_r = xt.rearrange("(kc p) s -> kc p s", p=128)
    wqt_r = wqt.rearrange("(kc p) n -> kc p n", p=128)
    wkt_r = wkt.rearrange("(kc p) n -> kc p n", p=128)
    wvt_r = wvt.rearrange("(kc p) n -> kc p n", p=128)
    wot_r = wot.rearrange("(hk p) n -> hk p n", p=128)

    with tile.TileContext(nc) as tc:
        with (
            tc.tile_pool(name="const", bufs=1) as constp,
            tc.tile_pool(name="persist", bufs=1) as persist,
        ):
            # constants
            cs1_sb = constp.tile([HD, S], F32)
            cs2_sb = constp.tile([HD, S], F32)
            mix1_sb = constp.tile([HD, HD], F32R)
            mix2_sb = constp.tile([HD, HD], F32R)
            onesc_sb = constp.tile([HD, 1], F32R)
            onesr_sb = constp.tile([1, HD], F32R)
            ident_sb = constp.tile([HD, HD], F32R)
            nc.sync.dma_start(out=cs1_sb[:], in_=cs1[:])
            nc.sync.dma_start(out=cs2_sb[:], in_=cs2[:])
            nc.sync.dma_start(out=mix1_sb[:], in_=mix1[:])
            nc.sync.dma_start(out=mix2_sb[:], in_=mix2[:])
            nc.sync.dma_start(out=onesc_sb[:], in_=onesc[:])
            nc.sync.dma_start(out=onesr_sb[:], in_=onesr[:])
            nc.sync.dma_start(out=ident_sb[:], in_=ident[:])

            # persistent activations
            qt_sb = persist.tile([128, N_QH, S], F32R)   # QT_rope
            kt_sb = persist.tile([128, S], F32R)         # KT_rope
            v_sb = persist.tile([128, N_KT, HD], F32R)   # V [tok-in-tile, kt, chan]

            # ---------------- phase 1: projections + rope ----------------
            with (
                tc.tile_pool(name="wq", bufs=1) as wqp,
                tc.tile_pool(name="xtp", bufs=6) as xtp,
                tc.tile_pool(name="uv", bufs=4) as uvp,
                tc.tile_pool(name="vt", bufs=2) as vtp,
                tc.tile_pool(name="p1q", bufs=1, space="PSUM") as p1q,
                tc.tile_pool(name="p1k", bufs=1, space="PSUM") as p1k,
                tc.tile_pool(name="p1r", bufs=1, space="PSUM") as p1r,
            ):
                wq_sb = wqp.tile([128, N_KC, 512], F32R)
                nc.sync.dma_start(out=wq_sb[:], in_=wqt_r[:].rearrange("kc p n -> p kc n"))
                wk_sb = wqp.tile([128, N_KC, HD], F32R)
                nc.sync.dma_start(out=wk_sb[:], in_=wkt_r[:].rearrange("kc p n -> p kc n"))
                wv_sb = wqp.tile([128, N_KC, HD], F32R)
                nc.sync.dma_start(out=wv_sb[:], in_=wvt_r[:].rearrange("kc p n -> p kc n"))

                for tcb in range(N_TC):
                    t0 = tcb * 512
                    scope = nc.named_scope(f"proj{tcb}"); scope.__enter__()
                    qps = [
                        p1q.tile([128, 512], F32, name=f"qps{tcb}_{h}", tag=f"qps{h}")
                        for h in range(N_QH)
                    ]
                    kps = p1k.tile([128, 512], F32, name=f"kps{tcb}", tag="kps")
                    vtps = p1k.tile([128, 512], F32, name=f"vtps{tcb}", tag="vtps")
                    for kc in range(N_KC):
                        xt_t = xtp.tile([128, 512], F32R, name=f"xt{tcb}_{kc}", tag="xt")
                        nc.sync.dma_start(out=xt_t[:], in_=xt_r[kc, :, t0:t0 + 512])
                        st, sp = kc == 0, kc == N_KC - 1
                        for h in range(N_QH):
                            nc.tensor.matmul(
                                qps[h][:], wq_sb[:, kc, h * 128:(h + 1) * 128],
                                xt_t[:], start=st, stop=sp,
                            )
                        nc.tensor.matmul(kps[:], wk_sb[:, kc, :], xt_t[:], start=st, stop=sp)
                        nc.tensor.matmul(vtps[:], wv_sb[:, kc, :], xt_t[:], start=st, stop=sp)

                    # rope for Q heads and K of this token chunk
                    for h in range(N_QH + 1):
                        src = kps if h == N_QH else qps[h]
                        u_t = uvp.tile([128, 512], F32R, name=f"u{tcb}_{h}", tag="u")
                        v_t = uvp.tile([128, 512], F32R, name=f"v{tcb}_{h}", tag="v")
                        nc.vector.tensor_mul(u_t[:], src[:], cs1_sb[:, t0:t0 + 512])
                        nc.vector.tensor_mul(v_t[:], src[:], cs2_sb[:, t0:t0 + 512])
                        rps = p1r.tile([128, 512], F32, name=f"rps{tcb}_{h}", tag="rps")
                        nc.tensor.matmul(rps[:], mix1_sb[:], u_t[:], start=True, stop=False)
                        nc.tensor.matmul(rps[:], mix2_sb[:], v_t[:], start=False, stop=True)
                        if h == N_QH:
                            nc.vector.tensor_copy(kt_sb[:, t0:t0 + 512], rps[:])
                        else:
                            nc.vector.tensor_copy(qt_sb[:, h, t0:t0 + 512], rps[:])

                    # V for this token chunk: VT -> PE transpose -> V
                    vt_sb = vtp.tile([128, 512], F32R, name=f"vts{tcb}", tag="vts")
                    nc.vector.tensor_copy(vt_sb[:], vtps[:])
                    vtr = p1r.tile([128, 4, 128], F32R, name=f"vtr{tcb}", tag="vtr")
                    for j in range(4):
                        nc.tensor.transpose(
                            vtr[:, j, :], vt_sb[:, j * 128:(j + 1) * 128],
                            ident_sb[:],
                        )
                    nc.vector.tensor_copy(v_sb[:, tcb * 4:(tcb + 1) * 4, :], vtr[:])
                    scope.__exit__(None, None, None)

            # ------------- phase 2+3: attention + AllGather per q-chunk -----
            # ------------- phase 4: wo GEMM per q-chunk ---------------------
            with (
                tc.tile_pool(name="wo", bufs=1) as wop,
                tc.tile_pool(name="ep", bufs=1) as ep,
                tc.tile_pool(name="zp", bufs=2) as zp,
                tc.tile_pool(name="np_", bufs=2) as np_,
                tc.tile_pool(name="agp", bufs=6) as agp,
                tc.tile_pool(name="fout", bufs=2) as foutp,
                tc.tile_pool(name="p2s", bufs=2, space="PSUM") as p2s,
                tc.tile_pool(name="p2pv", bufs=1, space="PSUM") as p2pv,
                tc.tile_pool(name="p2z", bufs=1, space="PSUM") as p2z,
                tc.tile_pool(name="p4f", bufs=1, space="PSUM") as p4f,
            ):
                wo_sb = wop.tile([128, N_KC, 512], F32R)
                nc.sync.dma_start(out=wo_sb[:], in_=wot_r[:].rearrange("hk p n -> p hk n"))

                for qc in range(N_TC):
                    q0 = qc * 512
                    scope = nc.named_scope(f"attn{qc}"); scope.__enter__()
                    for h in range(N_QH):
                        e_t = ep.tile([128, N_KT, 512], F32R, name=f"e{qc}_{h}", tag="e")
                        zpart = zp.tile([128, 512], F32, name=f"zp{qc}_{h}", tag="zpart")
                        pvps = p2pv.tile([128, 512], F32, name=f"pv{qc}_{h}", tag="pv")
                        for kt in range(N_KT):
                            k0 = kt * 128
                            stps = p2s.tile([128, 512], F32, name=f"st{qc}_{h}_{kt}", tag="st")
                            nc.tensor.matmul(
                                stps[:], kt_sb[:, k0:k0 + 128],
                                qt_sb[:, h, q0:q0 + 512], start=True, stop=True,
                            )
                            nc.scalar.activation(
                                out=e_t[:, kt, :], in_=stps[:],
                                func=mybir.ActivationFunctionType.Exp,
                            )
                            if kt == 0:
                                nc.vector.tensor_copy(zpart[:], e_t[:, 0, :])
                            else:
                                nc.vector.tensor_add(zpart[:], zpart[:], e_t[:, kt, :])
                            nc.tensor.matmul(
                                pvps[:], v_sb[:, kt, :], e_t[:, kt, :],
                                start=(kt == 0), stop=(kt == N_KT - 1),
                            )
                        zpr = zp.tile([128, 512], F32R, name=f"zpr{qc}_{h}", tag="zpr")
                        nc.vector.tensor_copy(zpr[:], zpart[:])
                        zps = p2z.tile([1, 512], F32, name=f"z{qc}_{h}", tag="zb")
                        nc.tensor.matmul(zps[:], onesc_sb[:], zpr[:], start=True, stop=True)
                        invz = zp.tile([1, 512], F32, name=f"iz{qc}_{h}", tag="invz")
                        nc.vector.reciprocal(invz[:], zps[:])
                        invzr = zp.tile([1, 512], F32R, name=f"izr{qc}_{h}", tag="invzr")
                        nc.vector.tensor_copy(invzr[:], invz[:])
                        bcps = p2z.tile([128, 512], F32, name=f"bc{qc}_{h}", tag="zb")
                        nc.tensor.matmul(bcps[:], onesr_sb[:], invzr[:], start=True, stop=True)
                        bc_sb = np_.tile([128, 512], F32, name=f"bcs{qc}_{h}", tag="bcs")
                        nc.vector.tensor_copy(bc_sb[:], bcps[:])
                        at_sb = np_.tile([128, 512], F32R, name=f"at{qc}_{h}", tag="at")
                        nc.vector.tensor_mul(at_sb[:], pvps[:], bc_sb[:])
                        nc.sync.dma_start(
                            out=ag_in[qc][h * 128:(h + 1) * 128, :], in_=at_sb[:]
                        )

                    scope.__exit__(None, None, None)
                    scope = nc.named_scope(f"ag{qc}"); scope.__enter__()
                    nc.gpsimd.collective_compute(
                        "AllGather",
                        mybir.AluOpType.bypass,
                        replica_groups=[list(range(N_CORES))],
                        ins=[ag_in[qc][:].opt()],
                        outs=[ag_out[qc][:].opt()],
                    )

                    scope.__exit__(None, None, None)
                    scope = nc.named_scope(f"wo{qc}"); scope.__enter__()
                    # wo GEMM for this q-chunk
                    fps = [
                        p4f.tile([128, 512], F32, name=f"f{qc}_{dt}", tag=f"f{dt}")
                        for dt in range(4)
                    ]
                    ago_r = ag_out[qc].rearrange("(hk p) n -> hk p n", p=128)
                    for hk in range(N_KC):
                        rhs_t = agp.tile([128, 512], F32R, name=f"ag{qc}_{hk}", tag="ag")
                        nc.gpsimd.dma_start(out=rhs_t[:], in_=ago_r[hk])
                        for dt in range(4):
                            nc.tensor.matmul(
                                fps[dt][:], wo_sb[:, hk, dt * 128:(dt + 1) * 128],
                                rhs_t[:], start=(hk == 0), stop=(hk == N_KC - 1),
                            )
                    for dt in range(4):
                        f_sb = foutp.tile([128, 512], F32, name=f"fs{qc}_{dt}", tag="fs")
                        nc.vector.tensor_copy(f_sb[:], fps[dt][:])
                        nc.gpsimd.dma_start(
                            out=out_ext[dt * 128:(dt + 1) * 128, q0:q0 + 512],
                            in_=f_sb[:],
                        )
                    scope.__exit__(None, None, None)

    _split_multi_waits(nc)
    return nc


def _host_prep(x, cos, sin, wq, wk, wv, wo):
    scale = np.float32(HD ** -0.5)
    perm = np.concatenate([np.arange(0, HD, 2), np.arange(1, HD, 2)])

    xT = np.ascontiguousarray(x.T)
    cosT = np.ascontiguousarray(cos.T)
    sinT = np.ascontiguousarray(sin.T)
    cs1 = np.concatenate([cosT, sinT], axis=0)
    cs2 = np.concatenate([sinT, cosT], axis=0)

    m1 = np.zeros((HD, HD), np.float32)
    m1[np.arange(64), np.arange(64)] = 1.0
    m1[np.arange(64) + 64, np.arange(64)] = -1.0
    m2 = np.zeros((HD, HD), np.float32)
    m2[np.arange(64), np.arange(64) + 64] = 1.0
    m2[np.arange(64) + 64, np.arange(64) + 64] = 1.0

    shared = {
        "xt": xT,
        "cs1": cs1,
        "cs2": cs2,
        "mix1": m1,
        "mix2": m2,
        "onesc": np.ones((HD, 1), np.float32),
        "onesr": np.ones((1, HD), np.float32),
        "ident": np.eye(HD, dtype=np.float32),
    }
    in_maps = []
    for c in range(N_CORES):
        wq_c = wq[c * 512:(c + 1) * 512].reshape(N_QH, HD, D)[:, perm, :]
        wq_c = (wq_c.reshape(512, D) * scale)
        wk_c = wk[c * HD:(c + 1) * HD][perm, :]
        wv_c = wv[c * HD:(c + 1) * HD]
        wo_c = wo[c * 512:(c + 1) * 512]
        m = dict(shared)
        m["wqt"] = np.ascontiguousarray(wq_c.T)
        m["wkt"] = np.ascontiguousarray(wk_c.T)
        m["wvt"] = np.ascontiguousarray(wv_c.T)
        m["wot"] = np.ascontiguousarray(wo_c.T)
        in_maps.append(m)
    return in_maps


def kernel(x, cos, sin, wq, wk, wv, wo, _trace=False):
    x = np.asarray(x, np.float32)
    cos = np.asarray(cos, np.float32)
    sin = np.asarray(sin, np.float32)
    wq = np.asarray(wq, np.float32)
    wk = np.asarray(wk, np.float32)
    wv = np.asarray(wv, np.float32)
    wo = np.asarray(wo, np.float32)

    in_maps = _host_prep(x, cos, sin, wq, wk, wv, wo)
    if "nc" not in _NC_CACHE:
        _NC_CACHE["nc"] = _build()
    nc = _NC_CACHE["nc"]
    res = run_bass_kernel_spmd(
        nc, in_maps, core_ids=list(range(N_CORES)), trace=_trace
    )
    finalT = np.concatenate([res.results[c]["out"] for c in range(N_CORES)], axis=0)
    out = np.ascontiguousarray(finalT.T, dtype=np.float32)
    if _trace:
        kernel._last_exec_time_ns = res.exec_time_ns
        kernel._last_result = res
    return out
